# revision 1
# baseline (speedup 1.0000x reference)
"""Bass/Trainium2 kernel for nn_Attention_6682969112611.

Math (faithful to the buggy torch module):
    k_t   = k.reshape(b, l, c)                  # row-major reshape, NOT a transpose
    score = (q @ k_t) / sqrt(l)                 # (b, c, c)
    score = softmax(score, axis=0)              # softmax over the BATCH axis
    out   = score @ v                           # (b, c, l)

B=16, C=2048, L=64. Sharding: the c (query-row) axis of q/score/out is split
across 8 cores (256 rows each); k and v are replicated. The batch-axis softmax
needs, for every (c, c') pair, all 16 batch values — which all live on the same
core under c-sharding, so there are no collectives. Different c' columns are
independent, so we stream over c' in chunks of 128 and accumulate the second
matmul in PSUM.

Per-core pipeline for each c' chunk j (128 columns):
  mm1   (PE, fp32):  scoreT[c',cq] = sum_l k_t[b,l,c'] q[b,cq,l]  -> PSUM
  exp   (ACT):       e = exp(scoreT * 0.125) -> SBUF bf16
  sum   (DVE/ACT):   d = sum_b e  (pairwise tree over the 16 batches)
  recip (DVE):       r = 1/d (fp32)
  mul   (DVE/ACT):   en[b] = e[b] * r  (bf16)
  mm2   (PE, bf16):  out[cq,l] += sum_c' en[c',cq] v[b,c',l] -> PSUM accum

Host-side layout prep (part of the sharding strategy, all cheap numpy):
  - q is pre-transposed to (l, b, c) so mm1 needs no on-chip transpose.
  - k is reshaped to its "buggy" k_t view and chunked along c'.
  - v is cast to bf16 and transposed to (c', b, l) so chunk loads are contiguous.
  - q/k are cast to bf16 (fp32 matmul streams at 1/4 rate on the PE).

Measured on the 8-core axon TRN2 terminal: ~123 us per-core HW exec,
L2 relative error vs the fp32 reference ~4.2e-3 (bf16 softmax weights / v;
fp32 PSUM accumulation everywhere). Engine occupancy at that point:
PE ~98us (pinned at 1.2 GHz cold clock - HAM never warms at this duty
cycle), DVE ~92us, ACT ~82us.
"""

import os

import numpy as np
import ml_dtypes

B, C, L = 16, 2048, 64
NCORES = 8
CB = C // NCORES  # 256 query rows per core
NJ = 16           # c' chunks of 128
P = 128

# debug bisect knobs (comma-separated): nopar, noacc, nomm2, norecip
_VARIANT = set(filter(None, os.environ.get("KERNEL_VARIANT", "").split(",")))

_NC_CACHE: dict = {}


def _build_nc():
    import concourse.mybir as mybir
    import concourse.tile as tile
    from concourse import bacc

    f32 = mybir.dt.float32
    bf16 = mybir.dt.bfloat16
    Exp = mybir.ActivationFunctionType.Exp
    ADD = mybir.AluOpType.add
    MUL = mybir.AluOpType.mult

    nc = bacc.Bacc(None, target_bir_lowering=False, debug=False)

    par = "par" in _VARIANT  # partition-64 matmul operands crash HW; default off
    if par:
        # qt[p, m, cq]: p = (b%2)*64 + l, m = b//2  (128, 8, 256) bf16
        qt = nc.declare_dram_parameter("qt", [P, 8, CB], bf16, isOutput=False)
        # kt[j, p, m, c']: p = (b%2)*64 + l, m = b//2  (16, 128, 8, 128) bf16
        kt = nc.declare_dram_parameter("kt", [NJ, P, 8, 128], bf16, isOutput=False)
    else:
        qt = nc.declare_dram_parameter("qt", [64, B, CB], bf16, isOutput=False)
        kt = nc.declare_dram_parameter("kt", [NJ, 64, B, 128], bf16, isOutput=False)
    # vt[j, c', b, l]  (16, 128, 16, 64) bf16
    vt = nc.declare_dram_parameter("vt", [NJ, P, B, L], bf16, isOutput=False)
    # outd[t, cq_lo, bq, h, l]: b = 4t + bq, cq = 128h + cq_lo  (4, 128, 4, 2, 64) fp32
    outd = nc.declare_dram_parameter("outd", [4, P, 4, 2, L], f32, isOutput=True)

    with tile.TileContext(nc) as tc:
        with (
            tc.tile_pool(name="qp", bufs=1) as qp,
            tc.tile_pool(name="kp", bufs=4) as kp,
            tc.tile_pool(name="vp", bufs=4) as vp,
            tc.tile_pool(name="ep", bufs=4) as ep,
            tc.tile_pool(name="enp", bufs=4) as enp,
            tc.tile_pool(name="tp", bufs=3) as tp,
            tc.tile_pool(name="dp", bufs=3) as dp,
            tc.tile_pool(name="osp", bufs=2) as osp,
            tc.tile_pool(name="mm1p", bufs=2, space="PSUM") as mm1p,
            tc.tile_pool(name="accp", bufs=1, space="PSUM") as accp,
        ):
            qt_s = qp.tile([P, 8, CB] if par else [64, B, CB], bf16)
            nbg = 4  # qt arrives in batch-groups; mm1 b-group s only needs part s
            for g in range(nbg):
                gs = (B // nbg) * g
                ge = (B // nbg) * (g + 1)
                nc.sync.dma_start(out=qt_s[:, gs:ge], in_=qt[:, gs:ge])

            accs = [accp.tile([P, 4, 2, L], f32, name=f"acc{t}") for t in range(4)]

            nwarm = int(os.environ.get("KERNEL_NWARM", "6"))
            if nwarm:
                wseed = qp.tile([P, 512], bf16, name="wseed")
                nc.vector.memset(wseed[:], 0)
                wps = mm1p.tile([P, 4, CB], f32, name="ps")
                for _ in range(nwarm):
                    nc.tensor.matmul(
                        wps[:, 0, :],
                        lhsT=wseed[:, :128],
                        rhs=wseed[:, :CB],
                        start=True,
                        stop=True,
                    )

            def emit_mm2_b(j, en_j, v_j, b):
                # PSUM accumulation: `start=True` clears has_written for the
                # whole 2KB bank, so only the FIRST matmul into each acc bank
                # may use it. The other j==0 slices then overwrite (their
                # bytes are still pending-zero) and j>0 accumulates.
                noacc = "noacc" in _VARIANT
                acc = accs[b // 4]
                for h in range(2):
                    first_in_bank = j == 0 and b % 4 == 0 and h == 0
                    last_in_bank = j == NJ - 1 and b % 4 == 3 and h == 1
                    nc.tensor.matmul(
                        acc[:, b % 4, h],
                        lhsT=en_j[:, b, h * 128 : (h + 1) * 128],
                        rhs=v_j[:, b],
                        start=True if noacc else first_in_bank,
                        stop=True if noacc else last_in_bank,
                        skip_group_check=(not noacc)
                        and not (first_in_bank or last_in_bank),
                    )

            def emit_mm2(j, en_j, v_j):
                for b in range(B):
                    emit_mm2_b(j, en_j, v_j, b)

            pending = None  # software pipeline: mm2 for chunk j-1 is emitted
            # during chunk j so the PE never waits on the softmax chain
            for j in range(NJ):
                k_j = kp.tile([P, 8, 128] if par else [64, B, 128], bf16, name="k_j")
                nc.sync.dma_start(out=k_j[:], in_=kt[j])
                v_j = vp.tile([P, B, L], bf16, name="v_j")
                nc.sync.dma_start(out=v_j[:], in_=vt[j])

                e_j = ep.tile([P, B, CB], bf16, name="e_j")
                for s in range(4):
                    ps = mm1p.tile([P, 4, CB], f32, name="ps")
                    for bi in range(4):
                        b = 4 * s + bi
                        if par:
                            pp, m = b % 2, b // 2
                            lhsT = k_j[pp * 64 : (pp + 1) * 64, m]
                            rhs = qt_s[pp * 64 : (pp + 1) * 64, m]
                        else:
                            lhsT = k_j[:, b]
                            rhs = qt_s[:, b]
                        nc.tensor.matmul(
                            ps[:, bi],
                            lhsT=lhsT,
                            rhs=rhs,
                            start=True,
                            stop=True,
                        )
                    nc.scalar.activation(
                        e_j[:, 4 * s : 4 * s + 4], ps[:], Exp, scale=0.125
                    )

                if pending is not None and "nomm2" not in _VARIANT:
                    emit_mm2(*pending)

                # sum over the 16 batches: pairwise tree. Big levels pinned to
                # DVE; small levels float (DVE/ACT per scheduler load).
                tteng = nc.any if "anytree" in _VARIANT else nc.vector
                t1 = tp.tile([P, 8, CB], bf16, name="t1")
                tteng.tensor_tensor(t1[:], e_j[:, 0:8], e_j[:, 8:16], ADD)
                t2 = tp.tile([P, 4, CB], bf16, name="t2")
                tteng.tensor_tensor(t2[:], t1[:, 0:4], t1[:, 4:8], ADD)
                t3 = tp.tile([P, 2, CB], bf16, name="t3")
                nc.any.tensor_tensor(t3[:], t2[:, 0:2], t2[:, 2:4], ADD)
                d_f = dp.tile([P, CB], f32, name="d_f")
                nc.any.tensor_tensor(d_f[:], t3[:, 0], t3[:, 1], ADD)
                r_f = dp.tile([P, CB], f32, name="r_f")
                if "norecip" in _VARIANT:
                    nc.vector.tensor_copy(out=r_f[:], in_=d_f[:])
                elif "slowrecip" in _VARIANT:
                    nc.vector.reciprocal(r_f[:], d_f[:])
                else:
                    nc.vector.reciprocal_approx_fast(r_f[:], d_f[:])
                r_b = dp.tile([P, CB], bf16, name="r_b")
                nc.vector.tensor_copy(out=r_b[:], in_=r_f[:])

                en_j = enp.tile([P, B, CB], bf16, name="en_j")
                if j == NJ - 1:
                    # tail: normalize in two halves, each followed by its mm2
                    # block; each acc bank's psum->sbuf copy + store DMA is
                    # emitted as soon as that bank's accumulation completes
                    for g in range(2):
                        nc.vector.tensor_tensor(
                            en_j[:, 8 * g : 8 * g + 8],
                            e_j[:, 8 * g : 8 * g + 8],
                            r_b[:, None, :].to_broadcast((P, 8, CB)),
                            MUL,
                        )
                        if "nomm2" not in _VARIANT:
                            for b in range(8 * g, 8 * g + 8):
                                emit_mm2_b(j, en_j, v_j, b)
                                if b % 4 == 3:
                                    t = b // 4
                                    o_s = osp.tile([P, 4, 2, L], f32, name="o_s")
                                    nc.any.tensor_copy(out=o_s[:], in_=accs[t][:])
                                    nc.sync.dma_start(out=outd[t], in_=o_s[:])
                else:
                    nc.vector.tensor_tensor(
                        en_j[:],
                        e_j[:],
                        r_b[:, None, :].to_broadcast((P, B, CB)),
                        MUL,
                    )
                    pending = (j, en_j, v_j)


    nc.compile()
    return nc


def get_nc():
    if "nc" not in _NC_CACHE:
        _NC_CACHE["nc"] = _build_nc()
    return _NC_CACHE["nc"]


def make_in_maps(q, k, v):
    q = np.asarray(q, dtype=np.float32)
    k = np.asarray(k, dtype=np.float32)
    v = np.asarray(v, dtype=np.float32)

    qb = q.astype(ml_dtypes.bfloat16)
    kb = k.astype(ml_dtypes.bfloat16)
    if "par" in _VARIANT:
        # q (b, cq, l) -> (par, l, m, cq) -> (128, 8, C); p = (b%2)*64 + l
        qt_all = np.ascontiguousarray(
            qb.reshape(8, 2, C, L).transpose(1, 3, 0, 2)
        ).reshape(P, 8, C)
        # k -> k_t[b, l, cfull] (row-major reshape) -> (j, par, l, m, c')
        ktt = np.ascontiguousarray(
            kb.reshape(8, 2, L, NJ, 128).transpose(3, 1, 2, 0, 4)
        ).reshape(NJ, P, 8, 128)
    else:
        qt_all = np.ascontiguousarray(qb.transpose(2, 0, 1))  # (l, b, c)
        ktt = np.ascontiguousarray(
            kb.reshape(B, L, NJ, 128).transpose(2, 1, 0, 3)
        )  # (j, l, b, c')
    # v -> bf16, (c', b, l) -> (j, c'128, b, l)
    vbt = np.ascontiguousarray(
        v.astype(ml_dtypes.bfloat16).transpose(1, 0, 2)
    ).reshape(NJ, P, B, L)

    in_maps = []
    for g in range(NCORES):
        in_maps.append(
            {
                "qt": np.ascontiguousarray(qt_all[:, :, g * CB : (g + 1) * CB]),
                "kt": ktt,
                "vt": vbt,
            }
        )
    return in_maps


def assemble_out(results):
    out = np.empty((B, C, L), dtype=np.float32)
    for g in range(NCORES):
        od = np.asarray(results[g]["outd"])  # (4, 128, 4, 2, 64)
        oc = od.transpose(0, 2, 3, 1, 4).reshape(B, CB, L)
        out[:, g * CB : (g + 1) * CB, :] = oc
    return out


def run(q, k, v, trace=False, trace_kwargs=None):
    """Run on 8 NeuronCores; returns (out, BassKernelResults)."""
    from concourse.bass_utils import run_bass_kernel_spmd

    nc = get_nc()
    in_maps = make_in_maps(q, k, v)
    kwargs = {}
    if trace:
        kwargs["trace"] = True
        if trace_kwargs:
            kwargs["trace_kwargs"] = trace_kwargs
    res = run_bass_kernel_spmd(nc, in_maps, core_ids=list(range(NCORES)), **kwargs)
    return assemble_out(res.results), res


def kernel(q, k, v):
    out, _ = run(q, k, v, trace=False)
    return out



# revision 2
# speedup vs baseline: 1.0046x; 1.0046x over previous
"""Bass/Trainium2 kernel for nn_Attention_6682969112611.

Math (faithful to the buggy torch module):
    k_t   = k.reshape(b, l, c)                  # row-major reshape, NOT a transpose
    score = (q @ k_t) / sqrt(l)                 # (b, c, c)
    score = softmax(score, axis=0)              # softmax over the BATCH axis
    out   = score @ v                           # (b, c, l)

B=16, C=2048, L=64. Sharding: the c (query-row) axis of q/score/out is split
across 8 cores (256 rows each); k and v are replicated. The batch-axis softmax
needs, for every (c, c') pair, all 16 batch values — which all live on the same
core under c-sharding, so there are no collectives. Different c' columns are
independent, so we stream over c' in chunks of 128 and accumulate the second
matmul in PSUM.

Per-core pipeline for each c' chunk j (128 columns):
  mm1   (PE, fp32):  scoreT[c',cq] = sum_l k_t[b,l,c'] q[b,cq,l]  -> PSUM
  exp   (ACT):       e = exp(scoreT * 0.125) -> SBUF bf16
  sum   (DVE/ACT):   d = sum_b e  (pairwise tree over the 16 batches)
  recip (DVE):       r = 1/d (fp32)
  mul   (DVE/ACT):   en[b] = e[b] * r  (bf16)
  mm2   (PE, bf16):  out[cq,l] += sum_c' en[c',cq] v[b,c',l] -> PSUM accum

Host-side layout prep (part of the sharding strategy, all cheap numpy):
  - q is pre-transposed to (l, b, c) so mm1 needs no on-chip transpose.
  - k is reshaped to its "buggy" k_t view and chunked along c'.
  - v is cast to bf16 and transposed to (c', b, l) so chunk loads are contiguous.
  - q/k are cast to bf16 (fp32 matmul streams at 1/4 rate on the PE).

Measured on the 8-core axon TRN2 terminal: ~123 us per-core HW exec,
L2 relative error vs the fp32 reference ~4.2e-3 (bf16 softmax weights / v;
fp32 PSUM accumulation everywhere). Engine occupancy at that point:
PE ~98us (pinned at 1.2 GHz cold clock - HAM never warms at this duty
cycle), DVE ~92us, ACT ~82us.
"""

import os

import numpy as np
import ml_dtypes

B, C, L = 16, 2048, 64
NCORES = 8
CB = C // NCORES  # 256 query rows per core
NJ = 16           # c' chunks of 128
P = 128

# debug bisect knobs (comma-separated): nopar, noacc, nomm2, norecip
_VARIANT = set(filter(None, os.environ.get("KERNEL_VARIANT", "").split(",")))

_NC_CACHE: dict = {}


def _build_nc():
    import concourse.mybir as mybir
    import concourse.tile as tile
    from concourse import bacc

    f32 = mybir.dt.float32
    bf16 = mybir.dt.bfloat16
    Exp = mybir.ActivationFunctionType.Exp
    ADD = mybir.AluOpType.add
    MUL = mybir.AluOpType.mult

    nc = bacc.Bacc(None, target_bir_lowering=False, debug=False)

    par = "par" in _VARIANT  # partition-64 matmul operands crash HW; default off
    if par:
        # qt[p, m, cq]: p = (b%2)*64 + l, m = b//2  (128, 8, 256) bf16
        qt = nc.declare_dram_parameter("qt", [P, 8, CB], bf16, isOutput=False)
        # kt[j, p, m, c']: p = (b%2)*64 + l, m = b//2  (16, 128, 8, 128) bf16
        kt = nc.declare_dram_parameter("kt", [NJ, P, 8, 128], bf16, isOutput=False)
    else:
        qt = nc.declare_dram_parameter("qt", [64, B, CB], bf16, isOutput=False)
        kt = nc.declare_dram_parameter("kt", [NJ, 64, B, 128], bf16, isOutput=False)
    # vt[j, c', b, l]  (16, 128, 16, 64) bf16
    vt = nc.declare_dram_parameter("vt", [NJ, P, B, L], bf16, isOutput=False)
    # outd[t, cq_lo, bq, h, l]: b = 4t + bq, cq = 128h + cq_lo  (4, 128, 4, 2, 64) fp32
    outd = nc.declare_dram_parameter("outd", [4, P, 4, 2, L], bf16, isOutput=True)

    with tile.TileContext(nc) as tc:
        with (
            tc.tile_pool(name="qp", bufs=1) as qp,
            tc.tile_pool(name="kp", bufs=4) as kp,
            tc.tile_pool(name="vp", bufs=4) as vp,
            tc.tile_pool(name="ep", bufs=4) as ep,
            tc.tile_pool(name="enp", bufs=4) as enp,
            tc.tile_pool(name="tp", bufs=3) as tp,
            tc.tile_pool(name="dp", bufs=3) as dp,
            tc.tile_pool(name="osp", bufs=2) as osp,
            tc.tile_pool(name="mm1p", bufs=2, space="PSUM") as mm1p,
            tc.tile_pool(name="accp", bufs=1, space="PSUM") as accp,
        ):
            qt_s = qp.tile([P, 8, CB] if par else [64, B, CB], bf16)
            k0_pre = kp.tile([P, 8, 128] if par else [64, B, 128], bf16, name="k_j")
            if not par:
                nc.sync.dma_start(out=k0_pre[:, 0:4], in_=kt[0][:, 0:4])
            nc.sync.dma_start(out=qt_s[:, 0:4], in_=qt[:, 0:4])
            if not par:
                nc.sync.dma_start(out=k0_pre[:, 4:16], in_=kt[0][:, 4:16])
            else:
                nc.sync.dma_start(out=k0_pre[:], in_=kt[0])
            for g in range(1, 4):
                gs = 4 * g
                nc.sync.dma_start(out=qt_s[:, gs : gs + 4], in_=qt[:, gs : gs + 4])

            accs = [accp.tile([P, 4, 2, L], f32, name=f"acc{t}") for t in range(4)]

            nwarm = int(os.environ.get("KERNEL_NWARM", "6"))
            if nwarm:
                wseed = qp.tile([P, 512], bf16, name="wseed")
                nc.vector.memset(wseed[:], 0)
                wps = mm1p.tile([P, 4, CB], f32, name="ps")
                for _ in range(nwarm):
                    nc.tensor.matmul(
                        wps[:, 0, :],
                        lhsT=wseed[:, :128],
                        rhs=wseed[:, :CB],
                        start=True,
                        stop=True,
                    )

            def emit_mm2_b(j, en_j, v_j, b):
                # PSUM accumulation: `start=True` clears has_written for the
                # whole 2KB bank, so only the FIRST matmul into each acc bank
                # may use it. The other j==0 slices then overwrite (their
                # bytes are still pending-zero) and j>0 accumulates.
                noacc = "noacc" in _VARIANT
                acc = accs[b // 4]
                for h in range(2):
                    first_in_bank = j == 0 and b % 4 == 0 and h == 0
                    last_in_bank = j == NJ - 1 and b % 4 == 3 and h == 1
                    nc.tensor.matmul(
                        acc[:, b % 4, h],
                        lhsT=en_j[:, b, h * 128 : (h + 1) * 128],
                        rhs=v_j[:, b],
                        start=True if noacc else first_in_bank,
                        stop=True if noacc else last_in_bank,
                        skip_group_check=(not noacc)
                        and not (first_in_bank or last_in_bank),
                    )

            def emit_mm2(j, en_j, v_j):
                for b in range(B):
                    emit_mm2_b(j, en_j, v_j, b)

            pending = None  # software pipeline: mm2 for chunk j-1 is emitted
            # during chunk j so the PE never waits on the softmax chain
            for j in range(NJ):
                if j == 0:
                    k_j = k0_pre
                else:
                    k_j = kp.tile([P, 8, 128] if par else [64, B, 128], bf16, name="k_j")
                    nc.sync.dma_start(out=k_j[:], in_=kt[j])
                v_j = vp.tile([P, B, L], bf16, name="v_j")
                nc.sync.dma_start(out=v_j[:], in_=vt[j])

                e_j = ep.tile([P, B, CB], bf16, name="e_j")
                for s in range(4):
                    ps = mm1p.tile([P, 4, CB], f32, name="ps")
                    for bi in range(4):
                        b = 4 * s + bi
                        if par:
                            pp, m = b % 2, b // 2
                            lhsT = k_j[pp * 64 : (pp + 1) * 64, m]
                            rhs = qt_s[pp * 64 : (pp + 1) * 64, m]
                        else:
                            lhsT = k_j[:, b]
                            rhs = qt_s[:, b]
                        nc.tensor.matmul(
                            ps[:, bi],
                            lhsT=lhsT,
                            rhs=rhs,
                            start=True,
                            stop=True,
                        )
                    nc.scalar.activation(
                        e_j[:, 4 * s : 4 * s + 4], ps[:], Exp, scale=0.125
                    )

                if pending is not None and "nomm2" not in _VARIANT:
                    emit_mm2(*pending)

                # sum over the 16 batches: pairwise tree. Big levels pinned to
                # DVE; small levels float (DVE/ACT per scheduler load).
                tteng = nc.any if "anytree" in _VARIANT else nc.vector
                t1 = tp.tile([P, 8, CB], bf16, name="t1")
                tteng.tensor_tensor(t1[:], e_j[:, 0:8], e_j[:, 8:16], ADD)
                t2 = tp.tile([P, 4, CB], bf16, name="t2")
                tteng.tensor_tensor(t2[:], t1[:, 0:4], t1[:, 4:8], ADD)
                t3 = tp.tile([P, 2, CB], bf16, name="t3")
                nc.any.tensor_tensor(t3[:], t2[:, 0:2], t2[:, 2:4], ADD)
                d_f = dp.tile([P, CB], f32, name="d_f")
                nc.any.tensor_tensor(d_f[:], t3[:, 0], t3[:, 1], ADD)
                r_f = dp.tile([P, CB], f32, name="r_f")
                if "norecip" in _VARIANT:
                    nc.vector.tensor_copy(out=r_f[:], in_=d_f[:])
                elif "slowrecip" in _VARIANT:
                    nc.vector.reciprocal(r_f[:], d_f[:])
                else:
                    nc.vector.reciprocal_approx_fast(r_f[:], d_f[:])
                r_b = dp.tile([P, CB], bf16, name="r_b")
                nc.vector.tensor_copy(out=r_b[:], in_=r_f[:])

                en_j = enp.tile([P, B, CB], bf16, name="en_j")
                if j == NJ - 1:
                    # tail: normalize in two halves, each followed by its mm2
                    # block; each acc bank's psum->sbuf copy + store DMA is
                    # emitted as soon as that bank's accumulation completes
                    for g in range(2):
                        nc.vector.tensor_tensor(
                            en_j[:, 8 * g : 8 * g + 8],
                            e_j[:, 8 * g : 8 * g + 8],
                            r_b[:, None, :].to_broadcast((P, 8, CB)),
                            MUL,
                        )
                        if "nomm2" not in _VARIANT:
                            for b in range(8 * g, 8 * g + 8):
                                emit_mm2_b(j, en_j, v_j, b)
                                if b % 4 == 3:
                                    t = b // 4
                                    o_s = osp.tile([P, 4, 2, L], bf16, name="o_s")
                                    nc.any.tensor_copy(out=o_s[:], in_=accs[t][:])
                                    nc.sync.dma_start(out=outd[t], in_=o_s[:])
                else:
                    nc.vector.tensor_tensor(
                        en_j[:],
                        e_j[:],
                        r_b[:, None, :].to_broadcast((P, B, CB)),
                        MUL,
                    )
                    pending = (j, en_j, v_j)


    nc.compile()
    return nc


def get_nc():
    if "nc" not in _NC_CACHE:
        _NC_CACHE["nc"] = _build_nc()
    return _NC_CACHE["nc"]


def make_in_maps(q, k, v):
    q = np.asarray(q, dtype=np.float32)
    k = np.asarray(k, dtype=np.float32)
    v = np.asarray(v, dtype=np.float32)

    qb = q.astype(ml_dtypes.bfloat16)
    kb = k.astype(ml_dtypes.bfloat16)
    if "par" in _VARIANT:
        # q (b, cq, l) -> (par, l, m, cq) -> (128, 8, C); p = (b%2)*64 + l
        qt_all = np.ascontiguousarray(
            qb.reshape(8, 2, C, L).transpose(1, 3, 0, 2)
        ).reshape(P, 8, C)
        # k -> k_t[b, l, cfull] (row-major reshape) -> (j, par, l, m, c')
        ktt = np.ascontiguousarray(
            kb.reshape(8, 2, L, NJ, 128).transpose(3, 1, 2, 0, 4)
        ).reshape(NJ, P, 8, 128)
    else:
        qt_all = np.ascontiguousarray(qb.transpose(2, 0, 1))  # (l, b, c)
        ktt = np.ascontiguousarray(
            kb.reshape(B, L, NJ, 128).transpose(2, 1, 0, 3)
        )  # (j, l, b, c')
    # v -> bf16, (c', b, l) -> (j, c'128, b, l)
    vbt = np.ascontiguousarray(
        v.astype(ml_dtypes.bfloat16).transpose(1, 0, 2)
    ).reshape(NJ, P, B, L)

    in_maps = []
    for g in range(NCORES):
        in_maps.append(
            {
                "qt": np.ascontiguousarray(qt_all[:, :, g * CB : (g + 1) * CB]),
                "kt": ktt,
                "vt": vbt,
            }
        )
    return in_maps


def assemble_out(results):
    out = np.empty((B, C, L), dtype=np.float32)
    for g in range(NCORES):
        od = np.asarray(results[g]["outd"])  # (4, 128, 4, 2, 64) bf16
        oc = od.astype(np.float32).transpose(0, 2, 3, 1, 4).reshape(B, CB, L)
        out[:, g * CB : (g + 1) * CB, :] = oc
    return out


def run(q, k, v, trace=False, trace_kwargs=None):
    """Run on 8 NeuronCores; returns (out, BassKernelResults)."""
    from concourse.bass_utils import run_bass_kernel_spmd

    nc = get_nc()
    in_maps = make_in_maps(q, k, v)
    kwargs = {}
    if trace:
        kwargs["trace"] = True
        if trace_kwargs:
            kwargs["trace_kwargs"] = trace_kwargs
    res = run_bass_kernel_spmd(nc, in_maps, core_ids=list(range(NCORES)), **kwargs)
    return assemble_out(res.results), res


def kernel(q, k, v):
    out, _ = run(q, k, v, trace=False)
    return out



# revision 3
# speedup vs baseline: 1.0195x; 1.0148x over previous
"""Bass/Trainium2 kernel for nn_Attention_6682969112611.

Math (faithful to the buggy torch module):
    k_t   = k.reshape(b, l, c)                  # row-major reshape, NOT a transpose
    score = (q @ k_t) / sqrt(l)                 # (b, c, c)
    score = softmax(score, axis=0)              # softmax over the BATCH axis
    out   = score @ v                           # (b, c, l)

B=16, C=2048, L=64. Sharding: the c (query-row) axis of q/score/out is split
across 8 cores (256 rows each); k and v are replicated. The batch-axis softmax
needs, for every (c, c') pair, all 16 batch values — which all live on the same
core under c-sharding, so there are no collectives. Different c' columns are
independent, so we stream over c' in chunks of 128 and accumulate the second
matmul in PSUM.

Per-core pipeline for each c' chunk j (128 columns):
  mm1   (PE, fp32):  scoreT[c',cq] = sum_l k_t[b,l,c'] q[b,cq,l]  -> PSUM
  exp   (ACT):       e = exp(scoreT * 0.125) -> SBUF bf16
  sum   (DVE/ACT):   d = sum_b e  (pairwise tree over the 16 batches)
  recip (DVE):       r = 1/d (fp32)
  mul   (DVE/ACT):   en[b] = e[b] * r  (bf16)
  mm2   (PE, bf16):  out[cq,l] += sum_c' en[c',cq] v[b,c',l] -> PSUM accum

Host-side layout prep (part of the sharding strategy, all cheap numpy):
  - q is pre-transposed to (l, b, c) so mm1 needs no on-chip transpose.
  - k is reshaped to its "buggy" k_t view and chunked along c'.
  - v is cast to bf16 and transposed to (c', b, l) so chunk loads are contiguous.
  - q/k are cast to bf16 (fp32 matmul streams at 1/4 rate on the PE).

Measured on the 8-core axon TRN2 terminal: ~123 us per-core HW exec,
L2 relative error vs the fp32 reference ~4.2e-3 (bf16 softmax weights / v;
fp32 PSUM accumulation everywhere). Engine occupancy at that point:
PE ~98us (pinned at 1.2 GHz cold clock - HAM never warms at this duty
cycle), DVE ~92us, ACT ~82us.
"""

import os

import numpy as np
import ml_dtypes

B, C, L = 16, 2048, 64
NCORES = 8
CB = C // NCORES  # 256 query rows per core
NJ = 16           # c' chunks of 128
P = 128

# debug bisect knobs (comma-separated): nopar, noacc, nomm2, norecip
_VARIANT = set(filter(None, os.environ.get("KERNEL_VARIANT", "").split(",")))

_NC_CACHE: dict = {}


def _build_nc():
    import concourse.mybir as mybir
    import concourse.tile as tile
    from concourse import bacc

    f32 = mybir.dt.float32
    bf16 = mybir.dt.bfloat16
    Exp = mybir.ActivationFunctionType.Exp
    ADD = mybir.AluOpType.add
    MUL = mybir.AluOpType.mult

    from concourse import hw_specs

    # The Tile list-scheduler plans with the cost model's ramped PE clock
    # (2.4GHz after 3us of busy). The real DVFS never ramps at this duty
    # cycle, so plans made for a fast PE serialize badly at 1.2GHz. Pin
    # the planning clock to the unramped p-state.
    hw_specs.TRN2Spec.PE_CYCLE = hw_specs.TRN2Spec.PE_CYCLE_PSTATE_MID

    nc = bacc.Bacc(None, target_bir_lowering=False, debug=False)

    par = "par" in _VARIANT  # partition-64 matmul operands crash HW; default off
    if par:
        # qt[p, m, cq]: p = (b%2)*64 + l, m = b//2  (128, 8, 256) bf16
        qt = nc.declare_dram_parameter("qt", [P, 8, CB], bf16, isOutput=False)
        # kt[j, p, m, c']: p = (b%2)*64 + l, m = b//2  (16, 128, 8, 128) bf16
        kt = nc.declare_dram_parameter("kt", [NJ, P, 8, 128], bf16, isOutput=False)
    else:
        qt = nc.declare_dram_parameter("qt", [64, B, CB], bf16, isOutput=False)
        kt = nc.declare_dram_parameter("kt", [NJ, 64, B, 128], bf16, isOutput=False)
    # vt[j, c', b, l]  (16, 128, 16, 64) bf16
    vt = nc.declare_dram_parameter("vt", [NJ, P, B, L], bf16, isOutput=False)
    # outd[t, cq_lo, bq, h, l]: b = 4t + bq, cq = 128h + cq_lo  (4, 128, 4, 2, 64) fp32
    outd = nc.declare_dram_parameter("outd", [4, P, 4, 2, L], bf16, isOutput=True)

    with tile.TileContext(nc) as tc:
        with (
            tc.tile_pool(name="qp", bufs=1) as qp,
            tc.tile_pool(name="kp", bufs=4) as kp,
            tc.tile_pool(name="vp", bufs=4) as vp,
            tc.tile_pool(name="ep", bufs=4) as ep,
            tc.tile_pool(name="enp", bufs=4) as enp,
            tc.tile_pool(name="tp", bufs=3) as tp,
            tc.tile_pool(name="dp", bufs=3) as dp,
            tc.tile_pool(name="osp", bufs=2) as osp,
            tc.tile_pool(name="mm1p", bufs=2, space="PSUM") as mm1p,
            tc.tile_pool(name="accp", bufs=1, space="PSUM") as accp,
        ):
            qt_s = qp.tile([P, 8, CB] if par else [64, B, CB], bf16)
            k0_pre = kp.tile([P, 8, 128] if par else [64, B, 128], bf16, name="k_j")
            if not par:
                nc.sync.dma_start(out=k0_pre[:, 0:4], in_=kt[0][:, 0:4])
            nc.sync.dma_start(out=qt_s[:, 0:4], in_=qt[:, 0:4])
            if not par:
                nc.sync.dma_start(out=k0_pre[:, 4:16], in_=kt[0][:, 4:16])
            else:
                nc.sync.dma_start(out=k0_pre[:], in_=kt[0])
            for g in range(1, 4):
                gs = 4 * g
                nc.sync.dma_start(out=qt_s[:, gs : gs + 4], in_=qt[:, gs : gs + 4])

            accs = [accp.tile([P, 4, 2, L], f32, name=f"acc{t}") for t in range(4)]

            nwarm = int(os.environ.get("KERNEL_NWARM", "6"))
            if nwarm:
                wseed = qp.tile([P, 512], bf16, name="wseed")
                nc.vector.memset(wseed[:], 0)
                wps = mm1p.tile([P, 4, CB], f32, name="ps")
                for _ in range(nwarm):
                    nc.tensor.matmul(
                        wps[:, 0, :],
                        lhsT=wseed[:, :128],
                        rhs=wseed[:, :CB],
                        start=True,
                        stop=True,
                    )

            def emit_mm2_b(j, en_j, v_j, b):
                # PSUM accumulation: `start=True` clears has_written for the
                # whole 2KB bank, so only the FIRST matmul into each acc bank
                # may use it. The other j==0 slices then overwrite (their
                # bytes are still pending-zero) and j>0 accumulates.
                noacc = "noacc" in _VARIANT
                acc = accs[b // 4]
                for h in range(2):
                    first_in_bank = j == 0 and b % 4 == 0 and h == 0
                    last_in_bank = j == NJ - 1 and b % 4 == 3 and h == 1
                    nc.tensor.matmul(
                        acc[:, b % 4, h],
                        lhsT=en_j[:, b, h * 128 : (h + 1) * 128],
                        rhs=v_j[:, b],
                        start=True if noacc else first_in_bank,
                        stop=True if noacc else last_in_bank,
                        skip_group_check=(not noacc)
                        and not (first_in_bank or last_in_bank),
                    )

            def emit_mm2(j, en_j, v_j):
                for b in range(B):
                    emit_mm2_b(j, en_j, v_j, b)

            pending = None  # software pipeline: mm2 for chunk j-1 is emitted
            # during chunk j so the PE never waits on the softmax chain
            for j in range(NJ):
                if j == 0:
                    k_j = k0_pre
                else:
                    k_j = kp.tile([P, 8, 128] if par else [64, B, 128], bf16, name="k_j")
                    nc.sync.dma_start(out=k_j[:], in_=kt[j])
                v_j = vp.tile([P, B, L], bf16, name="v_j")
                nc.sync.dma_start(out=v_j[:], in_=vt[j])

                e_j = ep.tile([P, B, CB], bf16, name="e_j")
                for s in range(4):
                    ps = mm1p.tile([P, 4, CB], f32, name="ps")
                    for bi in range(4):
                        b = 4 * s + bi
                        if par:
                            pp, m = b % 2, b // 2
                            lhsT = k_j[pp * 64 : (pp + 1) * 64, m]
                            rhs = qt_s[pp * 64 : (pp + 1) * 64, m]
                        else:
                            lhsT = k_j[:, b]
                            rhs = qt_s[:, b]
                        nc.tensor.matmul(
                            ps[:, bi],
                            lhsT=lhsT,
                            rhs=rhs,
                            start=True,
                            stop=True,
                        )
                    nc.scalar.activation(
                        e_j[:, 4 * s : 4 * s + 4], ps[:], Exp, scale=0.125
                    )

                if pending is not None and "nomm2" not in _VARIANT:
                    emit_mm2(*pending)

                # sum over the 16 batches: pairwise tree. Big levels pinned to
                # DVE; small levels float (DVE/ACT per scheduler load).
                tteng = nc.any if "anytree" in _VARIANT else nc.vector
                t1 = tp.tile([P, 8, CB], bf16, name="t1")
                tteng.tensor_tensor(t1[:], e_j[:, 0:8], e_j[:, 8:16], ADD)
                t2 = tp.tile([P, 4, CB], bf16, name="t2")
                tteng.tensor_tensor(t2[:], t1[:, 0:4], t1[:, 4:8], ADD)
                t3 = tp.tile([P, 2, CB], bf16, name="t3")
                nc.any.tensor_tensor(t3[:], t2[:, 0:2], t2[:, 2:4], ADD)
                d_f = dp.tile([P, CB], f32, name="d_f")
                nc.any.tensor_tensor(d_f[:], t3[:, 0], t3[:, 1], ADD)
                r_f = dp.tile([P, CB], f32, name="r_f")
                if "norecip" in _VARIANT:
                    nc.vector.tensor_copy(out=r_f[:], in_=d_f[:])
                elif "slowrecip" in _VARIANT:
                    nc.vector.reciprocal(r_f[:], d_f[:])
                else:
                    nc.vector.reciprocal_approx_fast(r_f[:], d_f[:])
                r_b = dp.tile([P, CB], bf16, name="r_b")
                nc.vector.tensor_copy(out=r_b[:], in_=r_f[:])

                en_j = enp.tile([P, B, CB], bf16, name="en_j")
                if j == NJ - 1:
                    # tail: normalize in two halves, each followed by its mm2
                    # block; each acc bank's psum->sbuf copy + store DMA is
                    # emitted as soon as that bank's accumulation completes
                    for g in range(2):
                        nc.vector.tensor_tensor(
                            en_j[:, 8 * g : 8 * g + 8],
                            e_j[:, 8 * g : 8 * g + 8],
                            r_b[:, None, :].to_broadcast((P, 8, CB)),
                            MUL,
                        )
                        if "nomm2" not in _VARIANT:
                            for b in range(8 * g, 8 * g + 8):
                                emit_mm2_b(j, en_j, v_j, b)
                                if b % 4 == 3:
                                    t = b // 4
                                    o_s = osp.tile([P, 4, 2, L], bf16, name="o_s")
                                    nc.any.tensor_copy(out=o_s[:], in_=accs[t][:])
                                    nc.sync.dma_start(out=outd[t], in_=o_s[:])
                else:
                    nc.vector.tensor_tensor(
                        en_j[:],
                        e_j[:],
                        r_b[:, None, :].to_broadcast((P, B, CB)),
                        MUL,
                    )
                    pending = (j, en_j, v_j)


    nc.compile()
    return nc


def get_nc():
    if "nc" not in _NC_CACHE:
        _NC_CACHE["nc"] = _build_nc()
    return _NC_CACHE["nc"]


def make_in_maps(q, k, v):
    q = np.asarray(q, dtype=np.float32)
    k = np.asarray(k, dtype=np.float32)
    v = np.asarray(v, dtype=np.float32)

    qb = q.astype(ml_dtypes.bfloat16)
    kb = k.astype(ml_dtypes.bfloat16)
    if "par" in _VARIANT:
        # q (b, cq, l) -> (par, l, m, cq) -> (128, 8, C); p = (b%2)*64 + l
        qt_all = np.ascontiguousarray(
            qb.reshape(8, 2, C, L).transpose(1, 3, 0, 2)
        ).reshape(P, 8, C)
        # k -> k_t[b, l, cfull] (row-major reshape) -> (j, par, l, m, c')
        ktt = np.ascontiguousarray(
            kb.reshape(8, 2, L, NJ, 128).transpose(3, 1, 2, 0, 4)
        ).reshape(NJ, P, 8, 128)
    else:
        qt_all = np.ascontiguousarray(qb.transpose(2, 0, 1))  # (l, b, c)
        ktt = np.ascontiguousarray(
            kb.reshape(B, L, NJ, 128).transpose(2, 1, 0, 3)
        )  # (j, l, b, c')
    # v -> bf16, (c', b, l) -> (j, c'128, b, l)
    vbt = np.ascontiguousarray(
        v.astype(ml_dtypes.bfloat16).transpose(1, 0, 2)
    ).reshape(NJ, P, B, L)

    in_maps = []
    for g in range(NCORES):
        in_maps.append(
            {
                "qt": np.ascontiguousarray(qt_all[:, :, g * CB : (g + 1) * CB]),
                "kt": ktt,
                "vt": vbt,
            }
        )
    return in_maps


def assemble_out(results):
    out = np.empty((B, C, L), dtype=np.float32)
    for g in range(NCORES):
        od = np.asarray(results[g]["outd"])  # (4, 128, 4, 2, 64) bf16
        oc = od.astype(np.float32).transpose(0, 2, 3, 1, 4).reshape(B, CB, L)
        out[:, g * CB : (g + 1) * CB, :] = oc
    return out


def run(q, k, v, trace=False, trace_kwargs=None):
    """Run on 8 NeuronCores; returns (out, BassKernelResults)."""
    from concourse.bass_utils import run_bass_kernel_spmd

    nc = get_nc()
    in_maps = make_in_maps(q, k, v)
    kwargs = {}
    if trace:
        kwargs["trace"] = True
        if trace_kwargs:
            kwargs["trace_kwargs"] = trace_kwargs
    res = run_bass_kernel_spmd(nc, in_maps, core_ids=list(range(NCORES)), **kwargs)
    return assemble_out(res.results), res


def kernel(q, k, v):
    out, _ = run(q, k, v, trace=False)
    return out



# revision 4
# speedup vs baseline: 1.0196x; 1.0001x over previous
"""Bass/Trainium2 kernel for nn_Attention_6682969112611.

Math (faithful to the buggy torch module):
    k_t   = k.reshape(b, l, c)                  # row-major reshape, NOT a transpose
    score = (q @ k_t) / sqrt(l)                 # (b, c, c)
    score = softmax(score, axis=0)              # softmax over the BATCH axis
    out   = score @ v                           # (b, c, l)

B=16, C=2048, L=64. Sharding: the c (query-row) axis of q/score/out is split
across 8 cores (256 rows each); k and v are replicated. The batch-axis softmax
needs, for every (c, c') pair, all 16 batch values — which all live on the same
core under c-sharding, so there are no collectives. Different c' columns are
independent, so we stream over c' in chunks of 128 and accumulate the second
matmul in PSUM.

Per-core pipeline for each c' chunk j (128 columns):
  mm1   (PE, fp32):  scoreT[c',cq] = sum_l k_t[b,l,c'] q[b,cq,l]  -> PSUM
  exp   (ACT):       e = exp(scoreT * 0.125) -> SBUF bf16
  sum   (DVE/ACT):   d = sum_b e  (pairwise tree over the 16 batches)
  recip (DVE):       r = 1/d (fp32)
  mul   (DVE/ACT):   en[b] = e[b] * r  (bf16)
  mm2   (PE, bf16):  out[cq,l] += sum_c' en[c',cq] v[b,c',l] -> PSUM accum

Host-side layout prep (part of the sharding strategy, all cheap numpy):
  - q is pre-transposed to (l, b, c) so mm1 needs no on-chip transpose.
  - k is reshaped to its "buggy" k_t view and chunked along c'.
  - v is cast to bf16 and transposed to (c', b, l) so chunk loads are contiguous.
  - q/k are cast to bf16 (fp32 matmul streams at 1/4 rate on the PE).

Measured on the 8-core axon TRN2 terminal: ~123 us per-core HW exec,
L2 relative error vs the fp32 reference ~4.2e-3 (bf16 softmax weights / v;
fp32 PSUM accumulation everywhere). Engine occupancy at that point:
PE ~98us (pinned at 1.2 GHz cold clock - HAM never warms at this duty
cycle), DVE ~92us, ACT ~82us.
"""

import os

import numpy as np
import ml_dtypes

B, C, L = 16, 2048, 64
NCORES = 8
CB = C // NCORES  # 256 query rows per core
NJ = 16           # c' chunks of 128
P = 128

# debug bisect knobs (comma-separated): nopar, noacc, nomm2, norecip
_VARIANT = set(filter(None, os.environ.get("KERNEL_VARIANT", "").split(",")))

_NC_CACHE: dict = {}


def _build_nc():
    import concourse.mybir as mybir
    import concourse.tile as tile
    from concourse import bacc

    f32 = mybir.dt.float32
    bf16 = mybir.dt.bfloat16
    Exp = mybir.ActivationFunctionType.Exp
    ADD = mybir.AluOpType.add
    MUL = mybir.AluOpType.mult

    from concourse import hw_specs

    # The Tile list-scheduler plans with the cost model's ramped PE clock
    # (2.4GHz after 3us of busy). The real DVFS never ramps at this duty
    # cycle, so plans made for a fast PE serialize badly at 1.2GHz. Pin
    # the planning clock to the unramped p-state.
    hw_specs.TRN2Spec.PE_CYCLE = hw_specs.TRN2Spec.PE_CYCLE_PSTATE_MID

    nc = bacc.Bacc(None, target_bir_lowering=False, debug=False)

    par = "par" in _VARIANT  # partition-64 matmul operands crash HW; default off
    if par:
        # qt[p, m, cq]: p = (b%2)*64 + l, m = b//2  (128, 8, 256) bf16
        qt = nc.declare_dram_parameter("qt", [P, 8, CB], bf16, isOutput=False)
        # kt[j, p, m, c']: p = (b%2)*64 + l, m = b//2  (16, 128, 8, 128) bf16
        kt = nc.declare_dram_parameter("kt", [NJ, P, 8, 128], bf16, isOutput=False)
    else:
        qt = nc.declare_dram_parameter("qt", [64, B, CB], bf16, isOutput=False)
        kt = nc.declare_dram_parameter("kt", [NJ, 64, B, 128], bf16, isOutput=False)
    # vt[j, c', b, l]  (16, 128, 16, 64) bf16
    vt = nc.declare_dram_parameter("vt", [NJ, P, B, L], bf16, isOutput=False)
    # outd[t, cq_lo, bq, h, l]: b = 4t + bq, cq = 128h + cq_lo  (4, 128, 4, 2, 64) fp32
    outd = nc.declare_dram_parameter("outd", [4, P, 4, 2, L], bf16, isOutput=True)

    with tile.TileContext(nc) as tc:
        with (
            tc.tile_pool(name="qp", bufs=1) as qp,
            tc.tile_pool(name="kp", bufs=4) as kp,
            tc.tile_pool(name="vp", bufs=4) as vp,
            tc.tile_pool(name="ep", bufs=4) as ep,
            tc.tile_pool(name="enp", bufs=4) as enp,
            tc.tile_pool(name="tp", bufs=3) as tp,
            tc.tile_pool(name="dp", bufs=3) as dp,
            tc.tile_pool(name="osp", bufs=4) as osp,
            tc.tile_pool(name="mm1p", bufs=2, space="PSUM") as mm1p,
            tc.tile_pool(name="accp", bufs=1, space="PSUM") as accp,
        ):
            qt_s = qp.tile([P, 8, CB] if par else [64, B, CB], bf16)
            k0_pre = kp.tile([P, 8, 128] if par else [64, B, 128], bf16, name="k_j")
            if not par:
                nc.sync.dma_start(out=k0_pre[:, 0:4], in_=kt[0][:, 0:4])
            nc.sync.dma_start(out=qt_s[:, 0:4], in_=qt[:, 0:4])
            if not par:
                nc.sync.dma_start(out=k0_pre[:, 4:16], in_=kt[0][:, 4:16])
            else:
                nc.sync.dma_start(out=k0_pre[:], in_=kt[0])
            for g in range(1, 4):
                gs = 4 * g
                nc.sync.dma_start(out=qt_s[:, gs : gs + 4], in_=qt[:, gs : gs + 4])

            accs = [accp.tile([P, 4, 2, L], f32, name=f"acc{t}") for t in range(4)]

            nwarm = int(os.environ.get("KERNEL_NWARM", "0"))
            if nwarm:
                wseed = qp.tile([P, 512], bf16, name="wseed")
                nc.vector.memset(wseed[:], 0)
                wps = mm1p.tile([P, 4, CB], f32, name="ps")
                for _ in range(nwarm):
                    nc.tensor.matmul(
                        wps[:, 0, :],
                        lhsT=wseed[:, :128],
                        rhs=wseed[:, :CB],
                        start=True,
                        stop=True,
                    )

            def emit_mm2_b(j, en_j, v_j, b):
                # PSUM accumulation: `start=True` clears has_written for the
                # whole 2KB bank, so only the FIRST matmul into each acc bank
                # may use it. The other j==0 slices then overwrite (their
                # bytes are still pending-zero) and j>0 accumulates.
                noacc = "noacc" in _VARIANT
                acc = accs[b // 4]
                for h in range(2):
                    first_in_bank = j == 0 and b % 4 == 0 and h == 0
                    last_in_bank = j == NJ - 1 and b % 4 == 3 and h == 1
                    nc.tensor.matmul(
                        acc[:, b % 4, h],
                        lhsT=en_j[:, b, h * 128 : (h + 1) * 128],
                        rhs=v_j[:, b],
                        start=True if noacc else first_in_bank,
                        stop=True if noacc else last_in_bank,
                        skip_group_check=(not noacc)
                        and not (first_in_bank or last_in_bank),
                    )

            def emit_mm2(j, en_j, v_j):
                for b in range(B):
                    emit_mm2_b(j, en_j, v_j, b)

            pending = None  # software pipeline: mm2 for chunk j-1 is emitted
            # during chunk j so the PE never waits on the softmax chain
            for j in range(NJ):
                if j == 0:
                    k_j = k0_pre
                else:
                    k_j = kp.tile([P, 8, 128] if par else [64, B, 128], bf16, name="k_j")
                    nc.sync.dma_start(out=k_j[:], in_=kt[j])
                v_j = vp.tile([P, B, L], bf16, name="v_j")
                nc.sync.dma_start(out=v_j[:], in_=vt[j])

                e_j = ep.tile([P, B, CB], bf16, name="e_j")
                for s in range(4):
                    ps = mm1p.tile([P, 4, CB], f32, name="ps")
                    for bi in range(4):
                        b = 4 * s + bi
                        if par:
                            pp, m = b % 2, b // 2
                            lhsT = k_j[pp * 64 : (pp + 1) * 64, m]
                            rhs = qt_s[pp * 64 : (pp + 1) * 64, m]
                        else:
                            lhsT = k_j[:, b]
                            rhs = qt_s[:, b]
                        nc.tensor.matmul(
                            ps[:, bi],
                            lhsT=lhsT,
                            rhs=rhs,
                            start=True,
                            stop=True,
                        )
                    nc.scalar.activation(
                        e_j[:, 4 * s : 4 * s + 4], ps[:], Exp, scale=0.125
                    )

                if pending is not None and "nomm2" not in _VARIANT:
                    emit_mm2(*pending)

                # sum over the 16 batches: pairwise tree. Big levels pinned to
                # DVE; small levels float (DVE/ACT per scheduler load).
                tteng = nc.any if "anytree" in _VARIANT else nc.vector
                t1 = tp.tile([P, 8, CB], bf16, name="t1")
                tteng.tensor_tensor(t1[:], e_j[:, 0:8], e_j[:, 8:16], ADD)
                t2 = tp.tile([P, 4, CB], bf16, name="t2")
                tteng.tensor_tensor(t2[:], t1[:, 0:4], t1[:, 4:8], ADD)
                t3 = tp.tile([P, 2, CB], bf16, name="t3")
                nc.any.tensor_tensor(t3[:], t2[:, 0:2], t2[:, 2:4], ADD)
                d_f = dp.tile([P, CB], f32, name="d_f")
                nc.any.tensor_tensor(d_f[:], t3[:, 0], t3[:, 1], ADD)
                r_f = dp.tile([P, CB], f32, name="r_f")
                if "norecip" in _VARIANT:
                    nc.vector.tensor_copy(out=r_f[:], in_=d_f[:])
                elif "slowrecip" in _VARIANT:
                    nc.vector.reciprocal(r_f[:], d_f[:])
                else:
                    nc.vector.reciprocal_approx_fast(r_f[:], d_f[:])
                r_b = dp.tile([P, CB], bf16, name="r_b")
                nc.vector.tensor_copy(out=r_b[:], in_=r_f[:])

                en_j = enp.tile([P, B, CB], bf16, name="en_j")
                if j == NJ - 1:
                    # tail: normalize in four quarters, each followed by its
                    # mm2 block; each acc bank's psum->sbuf copy + store DMA
                    # is emitted as soon as that bank's accumulation completes
                    for g in range(4):
                        nc.vector.tensor_tensor(
                            en_j[:, 4 * g : 4 * g + 4],
                            e_j[:, 4 * g : 4 * g + 4],
                            r_b[:, None, :].to_broadcast((P, 4, CB)),
                            MUL,
                        )
                        if "nomm2" not in _VARIANT:
                            for b in range(4 * g, 4 * g + 4):
                                emit_mm2_b(j, en_j, v_j, b)
                            o_s = osp.tile([P, 4, 2, L], bf16, name="o_s")
                            nc.any.tensor_copy(out=o_s[:], in_=accs[g][:])
                            nc.sync.dma_start(out=outd[g], in_=o_s[:])
                else:
                    nc.vector.tensor_tensor(
                        en_j[:],
                        e_j[:],
                        r_b[:, None, :].to_broadcast((P, B, CB)),
                        MUL,
                    )
                    pending = (j, en_j, v_j)


    nc.compile()
    return nc


def get_nc():
    if "nc" not in _NC_CACHE:
        _NC_CACHE["nc"] = _build_nc()
    return _NC_CACHE["nc"]


def make_in_maps(q, k, v):
    q = np.asarray(q, dtype=np.float32)
    k = np.asarray(k, dtype=np.float32)
    v = np.asarray(v, dtype=np.float32)

    qb = q.astype(ml_dtypes.bfloat16)
    kb = k.astype(ml_dtypes.bfloat16)
    if "par" in _VARIANT:
        # q (b, cq, l) -> (par, l, m, cq) -> (128, 8, C); p = (b%2)*64 + l
        qt_all = np.ascontiguousarray(
            qb.reshape(8, 2, C, L).transpose(1, 3, 0, 2)
        ).reshape(P, 8, C)
        # k -> k_t[b, l, cfull] (row-major reshape) -> (j, par, l, m, c')
        ktt = np.ascontiguousarray(
            kb.reshape(8, 2, L, NJ, 128).transpose(3, 1, 2, 0, 4)
        ).reshape(NJ, P, 8, 128)
    else:
        qt_all = np.ascontiguousarray(qb.transpose(2, 0, 1))  # (l, b, c)
        ktt = np.ascontiguousarray(
            kb.reshape(B, L, NJ, 128).transpose(2, 1, 0, 3)
        )  # (j, l, b, c')
    # v -> bf16, (c', b, l) -> (j, c'128, b, l)
    vbt = np.ascontiguousarray(
        v.astype(ml_dtypes.bfloat16).transpose(1, 0, 2)
    ).reshape(NJ, P, B, L)

    in_maps = []
    for g in range(NCORES):
        in_maps.append(
            {
                "qt": np.ascontiguousarray(qt_all[:, :, g * CB : (g + 1) * CB]),
                "kt": ktt,
                "vt": vbt,
            }
        )
    return in_maps


def assemble_out(results):
    out = np.empty((B, C, L), dtype=np.float32)
    for g in range(NCORES):
        od = np.asarray(results[g]["outd"])  # (4, 128, 4, 2, 64) bf16
        oc = od.astype(np.float32).transpose(0, 2, 3, 1, 4).reshape(B, CB, L)
        out[:, g * CB : (g + 1) * CB, :] = oc
    return out


def run(q, k, v, trace=False, trace_kwargs=None):
    """Run on 8 NeuronCores; returns (out, BassKernelResults)."""
    from concourse.bass_utils import run_bass_kernel_spmd

    nc = get_nc()
    in_maps = make_in_maps(q, k, v)
    kwargs = {}
    if trace:
        kwargs["trace"] = True
        if trace_kwargs:
            kwargs["trace_kwargs"] = trace_kwargs
    res = run_bass_kernel_spmd(nc, in_maps, core_ids=list(range(NCORES)), **kwargs)
    return assemble_out(res.results), res


def kernel(q, k, v):
    out, _ = run(q, k, v, trace=False)
    return out



# revision 5
# speedup vs baseline: 1.1039x; 1.0826x over previous
"""Bass/Trainium2 kernel for nn_Attention_6682969112611.

Math (faithful to the buggy torch module):
    k_t   = k.reshape(b, l, c)                  # row-major reshape, NOT a transpose
    score = (q @ k_t) / sqrt(l)                 # (b, c, c)
    score = softmax(score, axis=0)              # softmax over the BATCH axis
    out   = score @ v                           # (b, c, l)

B=16, C=2048, L=64. Sharding: the c (query-row) axis of q/score/out is split
across 8 cores (256 rows each); k and v are replicated. The batch-axis softmax
needs, for every (c, c') pair, all 16 batch values — which all live on the same
core under c-sharding, so there are no collectives. c' is streamed in 16 chunks
of 128; mm2 accumulates in 4 PSUM banks across chunks.

Per-core pipeline per c' chunk j:
  mm1 (PE, bf16):   scoreT[c',cq] per batch -> PSUM     (16 x 256-row matmuls)
  exp (ACT):        e = exp(scoreT * 0.125) -> SBUF bf16 (4 x [128,1024])
  sum (DVE):        d = sum_b e  (pairwise tree; first half emitted early,
                    overlapping the later EXPs)
  recip+cast (DVE): r = 1/d (fp32 fast-approx) -> bf16
  mul (DVE):        en = e * r (broadcast)
  mm2 (PE, bf16):   out[cq,l] += en^T v -> PSUM accum; emitted during chunk j+1

Tuning that matters (measured on the 8-core axon TRN2 terminal):
  - The Tile list-scheduler plans with the cost model's ramped 2.4GHz PE
    clock; the real DVFS never ramps at this kernel's duty cycle, so the
    planning clock is pinned to the unramped 1.2GHz p-state below.
  - Head: k_0 (one contiguous DMA) and qt group 0 are emitted first so the
    first mm1 waits on ~0.4MB, not the full prefetch stream.
  - Output is stored as bf16 and widened to fp32 on the host (halves the
    tail store traffic; adds ~2e-4 relative error).
  - Tail: the last chunk drains in 4 quarters (normalize -> mm2 -> psum
    copy -> store per accumulator bank), osp bufs=4 so copies never wait
    on store DMA buffer reuse.

Measured: ~120.1us per-core HW exec, L2 relative error vs the fp32
reference ~4.5e-3 (bf16 inputs/softmax weights/output; fp32 PSUM accum).
"""

import os

import numpy as np
import ml_dtypes

B, C, L = 16, 2048, 64
NCORES = 8
CB = C // NCORES  # 256 query rows per core
NJ = 16           # c' chunks of 128
P = 128

# debug bisect knobs (comma-separated): nopar, noacc, nomm2, norecip
_VARIANT = set(filter(None, os.environ.get("KERNEL_VARIANT", "").split(",")))

_NC_CACHE: dict = {}


def _build_nc():
    import concourse.mybir as mybir
    import concourse.tile as tile
    from concourse import bacc

    f32 = mybir.dt.float32
    bf16 = mybir.dt.bfloat16
    Exp = mybir.ActivationFunctionType.Exp
    ADD = mybir.AluOpType.add
    MUL = mybir.AluOpType.mult

    from concourse import hw_specs

    # The Tile list-scheduler plans with the cost model's ramped PE clock
    # (2.4GHz after 3us of busy). The real DVFS never ramps at this duty
    # cycle, so plans made for a fast PE serialize badly at 1.2GHz. Pin
    # the planning clock to the unramped p-state.
    hw_specs.TRN2Spec.PE_CYCLE = hw_specs.TRN2Spec.PE_CYCLE_PSTATE_MID

    nc = bacc.Bacc(None, target_bir_lowering=False, debug=False)

    par = "par" in _VARIANT  # partition-64 matmul operands crash HW; default off
    if par:
        # qt[p, m, cq]: p = (b%2)*64 + l, m = b//2  (128, 8, 256) bf16
        qt = nc.declare_dram_parameter("qt", [P, 8, CB], bf16, isOutput=False)
        # kt[j, p, m, c']: p = (b%2)*64 + l, m = b//2  (16, 128, 8, 128) bf16
        kt = nc.declare_dram_parameter("kt", [NJ, P, 8, 128], bf16, isOutput=False)
    else:
        qt = nc.declare_dram_parameter("qt", [64, B, CB], bf16, isOutput=False)
        kt = nc.declare_dram_parameter("kt", [NJ, 64, B, 128], bf16, isOutput=False)
    # vt[j, c', b, l]  (16, 128, 16, 64) bf16
    vt = nc.declare_dram_parameter("vt", [NJ, P, B, L], bf16, isOutput=False)
    # outd[t, cq_lo, bq, h, l]: b = 4t + bq, cq = 128h + cq_lo  (4, 128, 4, 2, 64) fp32
    outd = nc.declare_dram_parameter("outd", [4, P, 4, 2, L], bf16, isOutput=True)

    with tile.TileContext(nc) as tc:
        with (
            tc.tile_pool(name="qp", bufs=1) as qp,
            tc.tile_pool(name="kp", bufs=4) as kp,
            tc.tile_pool(name="vp", bufs=4) as vp,
            tc.tile_pool(name="ep", bufs=4) as ep,
            tc.tile_pool(name="enp", bufs=4) as enp,
            tc.tile_pool(name="tp", bufs=3) as tp,
            tc.tile_pool(name="dp", bufs=3) as dp,
            tc.tile_pool(name="osp", bufs=4) as osp,
            tc.tile_pool(name="mm1p", bufs=2, space="PSUM") as mm1p,
            tc.tile_pool(name="accp", bufs=1, space="PSUM") as accp,
        ):
            qt_s = qp.tile([P, 8, CB] if par else [64, B, CB], bf16)
            k0_pre = kp.tile([P, 8, 128] if par else [64, B, 128], bf16, name="k_j")
            nc.sync.dma_start(out=k0_pre[:], in_=kt[0])
            nc.sync.dma_start(out=qt_s[:, 0:4], in_=qt[:, 0:4])
            for g in range(1, 4):
                gs = 4 * g
                nc.sync.dma_start(out=qt_s[:, gs : gs + 4], in_=qt[:, gs : gs + 4])

            accs = [accp.tile([P, 4, 2, L], f32, name=f"acc{t}") for t in range(4)]

            nwarm = int(os.environ.get("KERNEL_NWARM", "0"))
            if nwarm:
                wseed = qp.tile([P, 512], bf16, name="wseed")
                nc.vector.memset(wseed[:], 0)
                wps = mm1p.tile([P, 4, CB], f32, name="ps")
                for _ in range(nwarm):
                    nc.tensor.matmul(
                        wps[:, 0, :],
                        lhsT=wseed[:, :128],
                        rhs=wseed[:, :CB],
                        start=True,
                        stop=True,
                    )

            def emit_mm2_b(j, en_j, v_j, b):
                # PSUM accumulation: `start=True` clears has_written for the
                # whole 2KB bank, so only the FIRST matmul into each acc bank
                # may use it. The other j==0 slices then overwrite (their
                # bytes are still pending-zero) and j>0 accumulates.
                noacc = "noacc" in _VARIANT
                acc = accs[b // 4]
                for h in range(2):
                    first_in_bank = j == 0 and b % 4 == 0 and h == 0
                    last_in_bank = j == NJ - 1 and b % 4 == 3 and h == 1
                    nc.tensor.matmul(
                        acc[:, b % 4, h],
                        lhsT=en_j[:, b, h * 128 : (h + 1) * 128],
                        rhs=v_j[:, b],
                        start=True if noacc else first_in_bank,
                        stop=True if noacc else last_in_bank,
                        skip_group_check=(not noacc)
                        and not (first_in_bank or last_in_bank),
                    )

            def emit_mm2(j, en_j, v_j):
                for b in range(B):
                    emit_mm2_b(j, en_j, v_j, b)

            pending = None  # software pipeline: mm2 for chunk j-1 is emitted
            # during chunk j so the PE never waits on the softmax chain
            for j in range(NJ):
                if j == 0:
                    k_j = k0_pre
                else:
                    k_j = kp.tile([P, 8, 128] if par else [64, B, 128], bf16, name="k_j")
                    nc.sync.dma_start(out=k_j[:], in_=kt[j])
                v_j = vp.tile([P, B, L], bf16, name="v_j")
                nc.sync.dma_start(out=v_j[:], in_=vt[j])

                e_j = ep.tile([P, B, CB], bf16, name="e_j")
                t1 = tp.tile([P, 8, CB], bf16, name="t1")
                for s in range(4):
                    ps = mm1p.tile([P, 4, CB], f32, name="ps")
                    for bi in range(4):
                        b = 4 * s + bi
                        if par:
                            pp, m = b % 2, b // 2
                            lhsT = k_j[pp * 64 : (pp + 1) * 64, m]
                            rhs = qt_s[pp * 64 : (pp + 1) * 64, m]
                        else:
                            lhsT = k_j[:, b]
                            rhs = qt_s[:, b]
                        nc.tensor.matmul(
                            ps[:, bi],
                            lhsT=lhsT,
                            rhs=rhs,
                            start=True,
                            stop=True,
                        )
                    nc.scalar.activation(
                        e_j[:, 4 * s : 4 * s + 4], ps[:], Exp, scale=0.125
                    )
                    if s == 1:
                        # A-half of the batch-sum runs while EXP2/3 stream
                        nc.vector.tensor_tensor(
                            t1[:, 0:4], e_j[:, 0:4], e_j[:, 4:8], ADD
                        )

                if pending is not None and "nomm2" not in _VARIANT:
                    emit_mm2(*pending)

                # sum over the 16 batches: pairwise tree (A emitted above)
                tteng = nc.any if "anytree" in _VARIANT else nc.vector
                tteng.tensor_tensor(t1[:, 4:8], e_j[:, 8:12], e_j[:, 12:16], ADD)
                t2 = tp.tile([P, 4, CB], bf16, name="t2")
                tteng.tensor_tensor(t2[:], t1[:, 0:4], t1[:, 4:8], ADD)
                t3 = tp.tile([P, 2, CB], bf16, name="t3")
                nc.any.tensor_tensor(t3[:], t2[:, 0:2], t2[:, 2:4], ADD)
                d_f = dp.tile([P, CB], f32, name="d_f")
                nc.any.tensor_tensor(d_f[:], t3[:, 0], t3[:, 1], ADD)
                r_f = dp.tile([P, CB], f32, name="r_f")
                if "norecip" in _VARIANT:
                    nc.vector.tensor_copy(out=r_f[:], in_=d_f[:])
                elif "slowrecip" in _VARIANT:
                    nc.vector.reciprocal(r_f[:], d_f[:])
                else:
                    nc.vector.reciprocal_approx_fast(r_f[:], d_f[:])
                r_b = dp.tile([P, CB], bf16, name="r_b")
                nc.vector.tensor_copy(out=r_b[:], in_=r_f[:])

                en_j = enp.tile([P, B, CB], bf16, name="en_j")
                if j == NJ - 1:
                    # tail: normalize in four quarters, each followed by its
                    # mm2 block; each acc bank's psum->sbuf copy + store DMA
                    # is emitted as soon as that bank's accumulation completes
                    for g in range(4):
                        nc.vector.tensor_tensor(
                            en_j[:, 4 * g : 4 * g + 4],
                            e_j[:, 4 * g : 4 * g + 4],
                            r_b[:, None, :].to_broadcast((P, 4, CB)),
                            MUL,
                        )
                        if "nomm2" not in _VARIANT:
                            for b in range(4 * g, 4 * g + 4):
                                emit_mm2_b(j, en_j, v_j, b)
                            o_s = osp.tile([P, 4, 2, L], bf16, name="o_s")
                            nc.any.tensor_copy(out=o_s[:], in_=accs[g][:])
                            nc.sync.dma_start(out=outd[g], in_=o_s[:])
                else:
                    nc.vector.tensor_tensor(
                        en_j[:],
                        e_j[:],
                        r_b[:, None, :].to_broadcast((P, B, CB)),
                        MUL,
                    )
                    pending = (j, en_j, v_j)


    nc.compile()
    return nc


def get_nc():
    if "nc" not in _NC_CACHE:
        _NC_CACHE["nc"] = _build_nc()
    return _NC_CACHE["nc"]


def make_in_maps(q, k, v):
    q = np.asarray(q, dtype=np.float32)
    k = np.asarray(k, dtype=np.float32)
    v = np.asarray(v, dtype=np.float32)

    qb = q.astype(ml_dtypes.bfloat16)
    kb = k.astype(ml_dtypes.bfloat16)
    if "par" in _VARIANT:
        # q (b, cq, l) -> (par, l, m, cq) -> (128, 8, C); p = (b%2)*64 + l
        qt_all = np.ascontiguousarray(
            qb.reshape(8, 2, C, L).transpose(1, 3, 0, 2)
        ).reshape(P, 8, C)
        # k -> k_t[b, l, cfull] (row-major reshape) -> (j, par, l, m, c')
        ktt = np.ascontiguousarray(
            kb.reshape(8, 2, L, NJ, 128).transpose(3, 1, 2, 0, 4)
        ).reshape(NJ, P, 8, 128)
    else:
        qt_all = np.ascontiguousarray(qb.transpose(2, 0, 1))  # (l, b, c)
        ktt = np.ascontiguousarray(
            kb.reshape(B, L, NJ, 128).transpose(2, 1, 0, 3)
        )  # (j, l, b, c')
    # v -> bf16, (c', b, l) -> (j, c'128, b, l)
    vbt = np.ascontiguousarray(
        v.astype(ml_dtypes.bfloat16).transpose(1, 0, 2)
    ).reshape(NJ, P, B, L)

    in_maps = []
    for g in range(NCORES):
        in_maps.append(
            {
                "qt": np.ascontiguousarray(qt_all[:, :, g * CB : (g + 1) * CB]),
                "kt": ktt,
                "vt": vbt,
            }
        )
    return in_maps


def assemble_out(results):
    out = np.empty((B, C, L), dtype=np.float32)
    for g in range(NCORES):
        od = np.asarray(results[g]["outd"])  # (4, 128, 4, 2, 64) bf16
        oc = od.astype(np.float32).transpose(0, 2, 3, 1, 4).reshape(B, CB, L)
        out[:, g * CB : (g + 1) * CB, :] = oc
    return out


def run(q, k, v, trace=False, trace_kwargs=None):
    """Run on 8 NeuronCores; returns (out, BassKernelResults)."""
    from concourse.bass_utils import run_bass_kernel_spmd

    nc = get_nc()
    in_maps = make_in_maps(q, k, v)
    kwargs = {}
    if trace:
        kwargs["trace"] = True
        if trace_kwargs:
            kwargs["trace_kwargs"] = trace_kwargs
    res = run_bass_kernel_spmd(nc, in_maps, core_ids=list(range(NCORES)), **kwargs)
    return assemble_out(res.results), res


def kernel(q, k, v):
    out, _ = run(q, k, v, trace=False)
    return out

